# revision 1
# baseline (speedup 1.0000x reference)
"""BloomBlock on 8 TRN2 NeuronCores.

Strategy (no device collectives — they have a ~60-100us floor on this part):
  * Launch 1 (L1): data-parallel over tokens. Each core owns 2 query blocks
    of 128 tokens (blocks i and 15-i — balances causal attention work).
    Computes LN1 (folded into weights on host), then Q^T,K^T (feature-major)
    and V (token-major) for its own 256 tokens, all in bf16.
  * Host: gathers K/V from all cores, builds per-core padded/permuted key
    layouts (slot 0..15 in natural chunk order). Masking and alibi are
    folded into per-(head,slot) exp-bias rows of kaug; out-of-prefix and
    pad keys get -1e9 so exp() underflows to exactly 0.
  * Launch 2 (L2): attention in transposed-score layout (keys on
    partitions, queries on free dim — softmax sum arrives free via an
    appended ones-column on V), then dense + residual, LN2, MLP, residual.
    K/V stream per head (small SBUF footprint) on the sync DMA queue while
    all MLP/dense weights prefetch on the Pool DMA queue from t=0.
    All matmuls bf16 with fp32 PSUM accumulation; residual path fp32.
"""

import os
from contextlib import ExitStack

import ml_dtypes
import numpy as np

import concourse.bass as bass
import concourse.tile as tile
from concourse import bacc, mybir
from concourse.masks import make_identity

BF16 = mybir.dt.bfloat16
F32 = mybir.dt.float32
NBF = ml_dtypes.bfloat16

S, H, NH, HD = 2048, 1024, 16, 64
NCORE = 8
QB = 128          # query/key chunk size
SC = 2 * QB       # tokens per core
NSLOT = 16        # padded key-chunk slots per core
NHC = H // 128    # hidden chunks
EPS = 1e-5
NORM = float(np.sqrt(HD))  # 8.0 (LAYER_NUMBER = 1)
NEG = -1e9


def _blocks(i):
    return (i, 15 - i)


def _slots(i):
    """Key-chunk id per slot for core i (None = zero pad). Diagonal chunks
    sit at fixed slots 14 (block A) and 15 (block B) so the post-exp mask
    multiplies land at the tail of the ctx accumulation chain."""
    a, b = _blocks(i)
    rest = [c for c in range(b + 1) if c not in (a, b)]
    s = rest + [None] * (NSLOT - 2 - len(rest))
    return s + [a, b]


# ----------------------------------------------------------------------------
# device kernels
# ----------------------------------------------------------------------------

def _layernorm_tiles(nc, tc, pools, x_tiles, out_all, ident, epst,
                     t_base=None):
    """x_tiles: list of [128, H] f32 sbuf tiles (token-major).
    Writes xhat^T (feature-major, bf16) into out_all; if t_base is None,
    out_all[c] is [128, len(x_tiles)*128] and tile t lands at cols
    [t*128:(t+1)*128]; with t_base given, out_all[c] is [128, 128] already
    sliced for this token tile."""
    stat, tp_ps, work = pools
    for t, xt in enumerate(x_tiles):
        st = stat.tile([128, 2, 6], F32, tag="bnst")
        nc.vector.bn_stats(out=st[:, 0, :], in_=xt[:, 0:512])
        nc.vector.bn_stats(out=st[:, 1, :], in_=xt[:, 512:1024])
        mv = stat.tile([128, 2], F32, tag="bnmv")
        nc.vector.bn_aggr(out=mv, in_=st)
        rstd = stat.tile([128, 1], F32, tag="rstd")
        nc.scalar.activation(out=rstd, in_=mv[:, 1:2],
                             func=mybir.ActivationFunctionType.Sqrt,
                             bias=epst, scale=1.0)
        nc.vector.reciprocal(out=rstd, in_=rstd)
        xh = work.tile([128, H], BF16, tag="xhat")
        nc.vector.tensor_scalar(out=xh, in0=xt, scalar1=mv[:, 0:1], scalar2=rstd,
                                op0=mybir.AluOpType.subtract,
                                op1=mybir.AluOpType.mult)
        for c in range(NHC):
            pst = tp_ps.tile([128, 128], BF16, tag="tp")
            nc.tensor.transpose(pst, xh[:, c * 128:(c + 1) * 128], ident)
            if t_base is None:
                dst = out_all[c][:, t * 128:(t + 1) * 128]
            else:
                dst = out_all[c]
            if t_base is not None:
                nc.scalar.activation(out=dst, in_=pst,
                                     func=mybir.ActivationFunctionType.Copy,
                                     bias=0.0, scale=1.0)
            else:
                nc.vector.tensor_copy(out=dst, in_=pst)


def build_l1():
    nc = bacc.Bacc("TRN2", target_bir_lowering=False, debug=False,
                   num_devices=NCORE)
    x = nc.dram_tensor("x", [SC, H], F32, kind="ExternalInput")
    # weights packed by host: row-chunk pairs side by side (fewer DMAs)
    wqkR = nc.dram_tensor("wqkR", [4, 128, 4 * H], BF16, kind="ExternalInput")
    wvR = nc.dram_tensor("wvR", [4, 128, 2 * H], BF16, kind="ExternalInput")
    bqkr = nc.dram_tensor("bqkr", [1, 2 * H], BF16, kind="ExternalInput")
    bvr = nc.dram_tensor("bvr", [1, H], BF16, kind="ExternalInput")
    # staged outputs, token-major: [t, 128, f]
    qkS = nc.dram_tensor("qkS", [2, 128, 2 * H], BF16, kind="ExternalOutput")
    vtS = nc.dram_tensor("vtS", [128, 2 * H], BF16, kind="ExternalOutput")

    with tile.TileContext(nc) as tc, ExitStack() as ctx:
        singles = ctx.enter_context(tc.tile_pool(name="singles", bufs=1))
        stat = ctx.enter_context(tc.tile_pool(name="stat", bufs=2))
        work = ctx.enter_context(tc.tile_pool(name="work", bufs=2))

        ident = singles.tile([128, 128], BF16)
        make_identity(nc, ident)
        epst = singles.tile([128, 1], F32)
        nc.vector.memset(epst, EPS)
        ones_row = singles.tile([1, 128], BF16)
        nc.vector.memset(ones_row, 1.0)
        # warm the Sqrt activation table while x streams in
        warm = singles.tile([1, 1], F32)
        nc.scalar.activation(out=warm, in_=epst[0:1, 0:1],
                             func=mybir.ActivationFunctionType.Sqrt,
                             bias=0.0, scale=1.0)
        # x in 4 half-tiles so bn_stats starts after the first 256 KB
        x_half = []
        for t in range(2):
            for hh in range(2):
                xt = singles.tile([128, 512], F32, tag=f"x{t}{hh}")
                nc.sync.dma_start(
                    out=xt, in_=x.ap()[t * 128:(t + 1) * 128,
                                       hh * 512:(hh + 1) * 512])
                x_half.append(xt)
        # weights behind x on the same queue (arrival order == use order)
        wqk_sb = []
        for g in range(4):
            wt = singles.tile([128, 4 * H], BF16, tag=f"wqk{g}")
            nc.sync.dma_start(out=wt, in_=wqkR.ap()[g])
            wqk_sb.append(wt)
        wv_sb = []
        for g in range(4):
            wt = singles.tile([128, 2 * H], BF16, tag=f"wv{g}")
            nc.sync.dma_start(out=wt, in_=wvR.ap()[g])
            wv_sb.append(wt)
        bqk_sb = singles.tile([128, 2 * H], BF16)
        nc.sync.dma_start(out=bqk_sb, in_=bqkr.ap().to_broadcast([128, 2 * H]))
        bv_sb = singles.tile([128, H], BF16)
        nc.sync.dma_start(out=bv_sb, in_=bvr.ap().to_broadcast([128, H]))
        qk_stage = singles.tile([128, 2, 2 * H], BF16)
        vt_stage = singles.tile([128, 2 * H], BF16)

        # LN over the half tiles (transpose pool is scoped so its PSUM
        # bank is free again before the QK accumulators open)
        xhatT = []
        for c in range(NHC):
            xc = singles.tile([128, SC], BF16, tag=f"xhatT{c}")
            xhatT.append(xc)
        def _ln_tile(t):
            st = stat.tile([128, 2, 6], F32, tag="bnst")
            nc.vector.bn_stats(out=st[:, 0, :], in_=x_half[2 * t])
            nc.vector.bn_stats(out=st[:, 1, :], in_=x_half[2 * t + 1])
            mv = stat.tile([128, 2], F32, tag="bnmv")
            nc.vector.bn_aggr(out=mv, in_=st)
            rstd = stat.tile([128, 1], F32, tag="rstd")
            nc.scalar.activation(out=rstd, in_=mv[:, 1:2],
                                 func=mybir.ActivationFunctionType.Sqrt,
                                 bias=epst, scale=1.0)
            nc.vector.reciprocal(out=rstd, in_=rstd)
            xh = work.tile([128, H], BF16, tag="xhat")
            for hh in range(2):
                nc.vector.tensor_scalar(
                    out=xh[:, hh * 512:(hh + 1) * 512],
                    in0=x_half[2 * t + hh],
                    scalar1=mv[:, 0:1], scalar2=rstd,
                    op0=mybir.AluOpType.subtract,
                    op1=mybir.AluOpType.mult)
            return xh

        def _tp_tile(tp_ps, t, xh):
            for c in range(NHC):
                pst = tp_ps.tile([128, 128], BF16, tag="tp")
                nc.tensor.transpose(pst, xh[:, c * 128:(c + 1) * 128],
                                    ident)
                nc.vector.tensor_copy(
                    out=xhatT[c][:, t * 128:(t + 1) * 128], in_=pst)

        with tc.tile_pool(name="tp_ps", bufs=2, space="PSUM") as tp_ps:
            # both LN chains first (DVE), then both transpose drains — the
            # DVE queue never makes transposes (and the QK behind them) wait
            xh0 = _ln_tile(0)
            xh1 = _ln_tile(1)
            _tp_tile(tp_ps, 0, xh0)
            _tp_tile(tp_ps, 1, xh1)

        # Q/K token-major [128 tok, 2H feats] per t-tile, both tiles
        # accumulating at once (8 banks), chunk-outer so each packed
        # weight tile is consumed the moment it lands; V reuses the same
        # psum tiles afterwards (no pool transition)
        with tc.tile_pool(name="qk_ps", bufs=1, space="PSUM") as qk_ps:
            # t-outer: t=0 accumulates while wqk streams in; its drain
            # overlaps t=1's full-speed pass, whose drain overlaps V, etc.
            # t=1's LN transposes are emitted between the passes so they
            # never head-of-line block the first QK matmuls.
            for t in range(2):
                psqk = qk_ps.tile([128, 2 * H], F32, tag=f"psqk{t}")
                for c in range(NHC):
                    wt = wqk_sb[c // 2]
                    for n in range(4):
                        nc.tensor.matmul(
                            psqk[:, n * 512:(n + 1) * 512],
                            lhsT=xhatT[c][:, t * 128:(t + 1) * 128],
                            rhs=wt[:, (c % 2) * 2 * H + n * 512:
                                   (c % 2) * 2 * H + (n + 1) * 512],
                            start=(c == 0), stop=(c == NHC - 1))
                for n in range(2):
                    sl = slice(n * 1024, (n + 1) * 1024)
                    nc.vector.tensor_add(out=qk_stage[:, t, sl],
                                         in0=psqk[:, sl], in1=bqk_sb[:, sl])
                nc.sync.dma_start(out=qkS.ap()[t], in_=qk_stage[:, t, :])

            # V token-major, chunk-outer behind the wv stream; reuses the
            # qk psum tags (t=0 bank frees while t=1 still accumulates)
            for t in range(2):
                psv = qk_ps.tile([128, 2 * H], F32, tag=f"psqk{t}")
                for c in range(NHC):
                    wt = wv_sb[c // 2]
                    for n in range(2):
                        nc.tensor.matmul(
                            psv[:, n * 512:(n + 1) * 512],
                            lhsT=xhatT[c][:, t * 128:(t + 1) * 128],
                            rhs=wt[:, (c % 2) * H + n * 512:
                                   (c % 2) * H + (n + 1) * 512],
                            start=(c == 0), stop=(c == NHC - 1))
                nc.vector.tensor_add(out=vt_stage[:, t * H:(t + 1) * H],
                                     in0=psv[:, 0:H], in1=bv_sb)
            nc.sync.dma_start(out=vtS.ap(), in_=vt_stage)
    nc.compile()
    return nc


def build_l2():
    nc = bacc.Bacc("TRN2", target_bir_lowering=False, debug=False,
                   num_devices=NCORE)
    # host pre-packs everything so each logical stream is ONE contiguous
    # dma per tile (hwdge has a ~625ns fixed cost per dma)
    qaug = nc.dram_tensor("qaug", [66, NH * SC], BF16, kind="ExternalInput")
    kaug = nc.dram_tensor("kaug", [NH, 66, NSLOT * QB], BF16, kind="ExternalInput")
    vaug = nc.dram_tensor("vaug", [NH, QB, NSLOT * 65], BF16, kind="ExternalInput")
    binm = nc.dram_tensor("binm", [QB, QB], BF16, kind="ExternalInput")
    xres = nc.dram_tensor("xres", [SC, H], F32, kind="ExternalInput")
    dwR = nc.dram_tensor("dwR", [2, 128, 4 * H], BF16, kind="ExternalInput")
    fc1a = nc.dram_tensor("fc1a", [4, 128, 4 * H], BF16, kind="ExternalInput")
    fc1b = nc.dram_tensor("fc1b", [4, 128, 4 * H], BF16, kind="ExternalInput")
    b1 = nc.dram_tensor("b1", [128, 32], F32, kind="ExternalInput")
    fc2R = nc.dram_tensor("fc2R", [8, 128, 4 * H], BF16, kind="ExternalInput")
    b2 = nc.dram_tensor("b2", [1, H], F32, kind="ExternalInput")
    # staged output, packed [128, t*H + f] (token t*128+p)
    outS = nc.dram_tensor("outS", [128, 2 * H], F32, kind="ExternalOutput")

    NB = 4  # key-chunk slots per exp batch

    with tile.TileContext(nc) as tc, ExitStack() as ctx:
        singles = ctx.enter_context(tc.tile_pool(name="singles", bufs=1))
        stat = ctx.enter_context(tc.tile_pool(name="stat", bufs=2))
        work = ctx.enter_context(tc.tile_pool(name="work", bufs=2))

        ident = singles.tile([128, 128], BF16)
        make_identity(nc, ident)
        epst = singles.tile([128, 1], F32)
        nc.vector.memset(epst, EPS)
        ones_col = singles.tile([1, 64], F32)
        nc.vector.memset(ones_col, 1.0)
        # warm the Exp table before the first scores arrive
        warm = singles.tile([1, 1], F32)
        nc.scalar.activation(out=warm, in_=epst[0:1, 0:1],
                             func=mybir.ActivationFunctionType.Exp,
                             bias=0.0, scale=1.0)

        # ---- attention-critical stream head ----
        bm = singles.tile([QB, QB], BF16)
        qaug_all = singles.tile([66, NH * SC], BF16)

        # weight tiles (DMAs are interleaved into the sync stream below,
        # in consumption order; everything lives on ONE queue so ordering
        # is deterministic and the k/v stream is never starved)
        dw_sb = []
        for g in range(2):
            wt = singles.tile([128, 4 * H], BF16, tag=f"dw{g}")
            dw_sb.append(wt)
        xres_t = []
        for t in range(2):
            xt = singles.tile([128, H], F32, tag=f"xres{t}")
            xres_t.append(xt)
        b1_sb = singles.tile([128, 32], F32)
        b2_sb = singles.tile([128, H], F32)
        # fc1 in two column halves (m<16 / m>=16), each as 4 packed tiles
        # of 2 row-chunks; first half preloads under the attention tail
        fc1a_sb = []
        fc1b_sb = []
        for g in range(4):
            wa = singles.tile([128, 4 * H], BF16, tag=f"fc1a{g}")
            fc1a_sb.append(wa)
            wb = singles.tile([128, 4 * H], BF16, tag=f"fc1b{g}")
            fc1b_sb.append(wb)

        def _fc1_slice(m, c):
            """lhsT slice for fc1 output tile m, contraction chunk c."""
            half = fc1a_sb if m < 16 else fc1b_sb
            return half[c // 2][:, (c % 2) * 2 * H + (m % 16) * 128:
                                (c % 2) * 2 * H + (m % 16) * 128 + 128]

        # normalized ctx (feature-major), written per head during attention
        ctxT = []
        for c in range(NHC):
            ct = singles.tile([128, SC], BF16, tag=f"ctxT{c}")
            ctxT.append(ct)
        f2p = ctx.enter_context(tc.tile_pool(name="fc2pool", bufs=3))
        fc2_sb = []

        # per-head interleave of bulk loads into the k/v stream: one
        # ~1MB group every second head fits in the dma slack without
        # re-pacing the attention (which is Act/exp-bound)
        def _bulk_loads(h):
            if h in (2, 4, 6, 8):
                g = (h - 2) // 2
                nc.sync.dma_start(out=fc1a_sb[g], in_=fc1a.ap()[g])
            elif h in (10, 12):
                g = (h - 10) // 2
                nc.sync.dma_start(out=dw_sb[g], in_=dwR.ap()[g])
            elif h == 14:
                nc.sync.dma_start(out=xres_t[0], in_=xres.ap()[0:128, :])

        # ------------------- attention -------------------
        with tc.tile_pool(name="kvstream", bufs=3) as kvp, \
             tc.tile_pool(name="probs", bufs=8) as ppool, \
             tc.tile_pool(name="sc_ps", bufs=2, space="PSUM") as sc_ps, \
             tc.tile_pool(name="rc_ps", bufs=2, space="PSUM") as rc_ps, \
             tc.tile_pool(name="ctx_ps", bufs=2, space="PSUM") as ctx_ps:

            def _norm_head(h, pctx):
                """softmax-normalize head h's raw ctx (runs one head late)"""
                c = h // 2
                p0 = 64 * (h % 2)
                recip = work.tile([1, SC], F32, tag="recip")
                nc.vector.reciprocal(out=recip, in_=pctx[64:65, :])
                prec = rc_ps.tile([64, SC], F32, tag="prec")
                nc.tensor.matmul(prec, lhsT=ones_col, rhs=recip,
                                 start=True, stop=True)
                recb = work.tile([64, SC], F32, tag="recb")
                nc.vector.tensor_copy(out=recb, in_=prec)
                nc.vector.tensor_mul(out=ctxT[c][p0:p0 + 64, :],
                                     in0=pctx[0:64, :], in1=recb)

            def _ctx_batch(pctx, vt, probs, b):
                if b == NSLOT // NB - 1:
                    # diagonal-chunk causal masks: slot 14 masks the
                    # A-half, slot 15 the B-half (binary mult, post-exp)
                    j14 = (NSLOT - 2) % NB
                    nc.vector.tensor_mul(
                        out=probs[:, j14 * SC: j14 * SC + QB],
                        in0=probs[:, j14 * SC: j14 * SC + QB], in1=bm)
                    j15 = (NSLOT - 1) % NB
                    nc.vector.tensor_mul(
                        out=probs[:, j15 * SC + QB:(j15 + 1) * SC],
                        in0=probs[:, j15 * SC + QB:(j15 + 1) * SC], in1=bm)
                for j in range(NB):
                    s = b * NB + j
                    nc.tensor.matmul(
                        pctx,
                        lhsT=vt[:, s * 65:(s + 1) * 65],
                        rhs=probs[:, j * SC:(j + 1) * SC],
                        start=(s == 0), stop=(s == NSLOT - 1))

            # ctx stream runs one batch behind the scores stream so that
            # neither PE (ctx waits exp) nor Act (exp waits scores at head
            # boundaries) ever stalls the other
            # h=0's k/v go first on the queue (they gate the first scores),
            # then qaug, then the mask tile
            kt0 = kvp.tile([66, NSLOT * QB], BF16, tag="k")
            nc.sync.dma_start(out=kt0, in_=kaug.ap()[0])
            nc.sync.dma_start(out=qaug_all, in_=qaug.ap())
            vt0 = kvp.tile([QB, NSLOT * 65], BF16, tag="v")
            nc.sync.dma_start(out=vt0, in_=vaug.ap()[0])
            nc.sync.dma_start(out=bm, in_=binm.ap())

            pend = None
            pend_ctx = None
            for h in range(NH):
                if h == 0:
                    kt, vt = kt0, vt0
                else:
                    kt = kvp.tile([66, NSLOT * QB], BF16, tag="k")
                    nc.sync.dma_start(out=kt, in_=kaug.ap()[h])
                    vt = kvp.tile([QB, NSLOT * 65], BF16, tag="v")
                    nc.sync.dma_start(out=vt, in_=vaug.ap()[h])
                _bulk_loads(h)

                pctx = ctx_ps.tile([65, SC], F32, tag="pctx")
                for b in range(NSLOT // NB):
                    psb = sc_ps.tile([128, NB * SC], F32, tag="psb")
                    for j in range(NB):
                        s = b * NB + j
                        nc.tensor.matmul(
                            psb[:, j * SC:(j + 1) * SC],
                            lhsT=kt[:, s * QB:(s + 1) * QB],
                            rhs=qaug_all[:, h * SC:(h + 1) * SC],
                            start=True, stop=True)
                    if pend_ctx is not None:
                        _ctx_batch(*pend_ctx)
                    probs = ppool.tile([128, NB * SC], BF16, tag="probs")
                    nc.scalar.activation(out=probs, in_=psb,
                                         func=mybir.ActivationFunctionType.Exp,
                                         bias=0.0, scale=1.0)
                    if b == 2 and pend is not None:
                        _norm_head(*pend)
                        pend = None
                    pend_ctx = (pctx, vt, probs, b)
                pend = (h, pctx)
            _ctx_batch(*pend_ctx)
            _norm_head(*pend)

        # swap Exp -> Gelu table while dense/LN2 run on other engines
        nc.scalar.activation(out=warm, in_=epst[0:1, 0:1],
                             func=mybir.ActivationFunctionType.Gelu_apprx_tanh,
                             bias=0.0, scale=1.0)

        # ---- post-attention loads: residual tail, biases, then the fc2
        # stream interleaved with the second fc1 half, consumption order ----
        nc.sync.dma_start(out=xres_t[1], in_=xres.ap()[128:256, :])
        nc.sync.dma_start(out=b1_sb, in_=b1.ap())
        nc.sync.dma_start(out=b2_sb, in_=b2.ap().to_broadcast([128, H]))
        for g in range(8):
            wt = f2p.tile([128, 4 * H], BF16, tag="fc2")
            nc.sync.dma_start(out=wt, in_=fc2R.ap()[g])
            fc2_sb.append(wt)
            if g < 4:
                nc.sync.dma_start(out=fc1b_sb[g], in_=fc1b.ap()[g])

        # ------------- dense + residual (t-major) + LN2 per t -------------
        # xh2T split per token tile: fc1 half-chains for t=0 can start
        # while LN2 for t=1 is still on the DVE
        xh2T = [[], []]
        for t in range(2):
            for c in range(NHC):
                xc = singles.tile([128, 128], BF16, tag=f"xh2T{t}{c}")
                xh2T[t].append(xc)
        with tc.tile_pool(name="mm2_ps", bufs=1, space="PSUM") as mm2_ps, \
             tc.tile_pool(name="tp2_ps", bufs=2, space="PSUM") as tp2_ps:
            # both dense passes back-to-back on PE (no LN2 transposes in
            # between — those would head-of-line block the PE queue while
            # waiting on the DVE bn/xhat chain); residual adds on Pool so
            # the DVE queue is free for LN2 stats immediately
            for t in range(2):
                psd = []
                for n in range(2):
                    psd_tn = mm2_ps.tile([128, 512], F32, tag=f"psd{t}{n}")
                    psd.append(psd_tn)
                for c in range(NHC):
                    dwc = dw_sb[c // 4]
                    co = (c % 4) * H
                    for n in range(2):
                        nc.tensor.matmul(
                            psd[n],
                            lhsT=ctxT[c][:, t * 128:(t + 1) * 128],
                            rhs=dwc[:, co + n * 512:co + (n + 1) * 512],
                            start=(c == 0), stop=(c == NHC - 1))
                for n in range(2):
                    sl = slice(n * 512, (n + 1) * 512)
                    # attn_out = dense + (x + dense_bias), in place
                    # (gpsimd cannot read PSUM on hw, so DVE does both)
                    nc.vector.tensor_add(out=xres_t[t][:, sl],
                                         in0=psd[n], in1=xres_t[t][:, sl])
            for t in range(2):
                _layernorm_tiles(nc, tc, (stat, tp2_ps, work),
                                 [xres_t[t]], xh2T[t],
                                 ident, epst, t_base=t)
        # residual2 also carries the fc2 bias; add it once LN2 has
        # consumed xres (runs on DVE under the MLP's shadow)
        for t in range(2):
            nc.vector.tensor_add(out=xres_t[t], in0=xres_t[t], in1=b2_sb)
        # make sure the gelu table is resident before the first fc1 drain
        nc.scalar.activation(out=warm, in_=epst[0:1, 0:1],
                             func=mybir.ActivationFunctionType.Gelu_apprx_tanh,
                             bias=0.0, scale=1.0)

        # ------------------- MLP (fused fc1 -> gelu -> fc2) -------------------
        with tc.tile_pool(name="hpool", bufs=6) as hp, \
             tc.tile_pool(name="mm3_ps", bufs=3, space="PSUM") as mm_ps:
            psf2 = {}
            for t in range(2):
                for n in range(2):
                    pf2 = mm_ps.tile([128, 512], F32, tag=f"psf2{t}{n}", bufs=1)
                    psf2[(t, n)] = pf2
            hts = {}
            for mm_i in range(33):
                if mm_i < 32:
                    m = mm_i
                    ps = mm_ps.tile([128, SC], F32, tag="psf1", bufs=4)
                    for t in range(2):
                        for c in range(NHC):
                            nc.tensor.matmul(
                                ps[:, t * 128:(t + 1) * 128],
                                lhsT=_fc1_slice(m, c),
                                rhs=xh2T[t][c],
                                start=(c == 0), stop=(c == NHC - 1))
                    ht = hp.tile([128, SC], BF16, tag="hT")
                    if os.environ.get("BLOOM_SIM") or os.environ.get("BLOOM_GELU_CHAIN"):
                        # bloom gelu: u*0.5*(1+tanh(0.79788456*u*(1+0.044715*u^2)))
                        # (explicit chain — CoreSim has no Gelu_apprx_tanh)
                        u = work.tile([128, SC], F32, tag="gelu_u")
                        nc.vector.tensor_scalar_add(out=u, in0=ps,
                                                    scalar1=b1_sb[:, m:m + 1])
                        s2 = work.tile([128, SC], F32, tag="gelu_s")
                        nc.vector.tensor_mul(out=s2, in0=u, in1=u)
                        nc.vector.tensor_scalar(out=s2, in0=s2,
                                                scalar1=0.035677408145115,
                                                scalar2=0.7978845608028654,
                                                op0=mybir.AluOpType.mult,
                                                op1=mybir.AluOpType.add)
                        nc.vector.tensor_mul(out=s2, in0=s2, in1=u)
                        nc.scalar.activation(out=s2, in_=s2,
                                             func=mybir.ActivationFunctionType.Tanh,
                                             bias=0.0, scale=1.0)
                        nc.vector.tensor_scalar(out=s2, in0=s2, scalar1=1.0,
                                                scalar2=0.5,
                                                op0=mybir.AluOpType.add,
                                                op1=mybir.AluOpType.mult)
                        nc.vector.tensor_mul(out=ht, in0=s2, in1=u)
                    else:
                        nc.scalar.activation(
                            out=ht, in_=ps,
                            func=mybir.ActivationFunctionType.Gelu_apprx_tanh,
                            bias=b1_sb[:, m:m + 1], scale=1.0)
                    hts[m] = ht
                if mm_i >= 1:
                    m = mm_i - 1
                    ht = hts.pop(m)
                    f2 = fc2_sb[m // 4]
                    f2o = (m % 4) * H
                    for t in range(2):
                        for n in range(2):
                            nc.tensor.matmul(
                                psf2[(t, n)],
                                lhsT=ht[:, t * 128:(t + 1) * 128],
                                rhs=f2[:, f2o + n * 512:f2o + (n + 1) * 512],
                                start=(m == 0), stop=(m == 31))
            out_stage = hp.tile([128, 2 * H], F32, tag="ostage", bufs=1)
            for t in range(2):
                for n in range(2):
                    sl = slice(n * 512, (n + 1) * 512)
                    nc.vector.tensor_add(
                        out=out_stage[:, t * H + n * 512:t * H + (n + 1) * 512],
                        in0=psf2[(t, n)], in1=xres_t[t][:, sl])
                # ship each token-tile half as soon as its adds land
                nc.sync.dma_start(out=outS.ap()[:, t * H:(t + 1) * H],
                                  in_=out_stage[:, t * H:(t + 1) * H])
    nc.compile()
    return nc


# ----------------------------------------------------------------------------
# host orchestration
# ----------------------------------------------------------------------------

_NC_CACHE = {}


def _get_nc(name):
    if name not in _NC_CACHE:
        _NC_CACHE[name] = build_l1() if name == "l1" else build_l2()
    return _NC_CACHE[name]


def _run(nc, in_maps):
    if os.environ.get("BLOOM_SIM"):
        from concourse.bass_interp import CoreSim
        results = []
        for m in in_maps:
            sim = CoreSim(nc, trace=False)
            for k, v in m.items():
                sim.tensor(k)[:] = v
            sim.simulate(check_with_hw=False)
            outs = {}
            for alloc in nc.m.functions[0].allocations:
                if getattr(alloc, "kind", None) == "ExternalOutput":
                    nm = alloc.memorylocations[0].name
                    outs[nm] = np.array(sim.tensor(nm))
            results.append(outs)
        return results
    from concourse.bass_utils import run_bass_kernel_spmd
    res = run_bass_kernel_spmd(nc, in_maps, core_ids=list(range(NCORE)))
    return res.results


def _prep_weights(ln1_g, ln1_b, qkv_w, qkv_b, dense_w, dense_b,
                  ln2_g, ln2_b, fc1_w, fc1_b, fc2_w, fc2_b):
    qkv_w = np.asarray(qkv_w, np.float32)
    qkv_b = np.asarray(qkv_b, np.float32)
    w_eff = qkv_w * np.asarray(ln1_g, np.float32)[None, :]
    b_eff = qkv_b + qkv_w @ np.asarray(ln1_b, np.float32)
    w3 = w_eff.reshape(NH, 3 * HD, H)
    b3 = b_eff.reshape(NH, 3 * HD)
    wq = w3[:, :HD, :] / NORM
    wk = w3[:, HD:2 * HD, :]
    wv = w3[:, 2 * HD:, :]
    bq = b3[:, :HD] / NORM
    bk = b3[:, HD:2 * HD]
    bv = b3[:, 2 * HD:]
    wqk = np.concatenate([wq.reshape(H, H), wk.reshape(H, H)], 0)  # [2H, H]
    wqkT = np.ascontiguousarray(wqk.T).astype(NBF)                  # [H, 2H]
    wvT = np.ascontiguousarray(wv.reshape(H, H).T).astype(NBF)      # [H, H]
    bqk = np.concatenate([bq.reshape(H), bk.reshape(H)])            # [2H]
    bqk_r = np.ascontiguousarray(bqk.reshape(1, 2 * H)).astype(NBF)
    bv_r = np.ascontiguousarray(bv.reshape(1, H)).astype(NBF)

    def pack_rows(w, group):
        """[R*128, C] -> [R//group, 128, group*C]: row-chunks side by side."""
        r, c = w.shape[0] // 128, w.shape[1]
        return np.ascontiguousarray(
            w.reshape(r // group, group, 128, c).transpose(0, 2, 1, 3)
            .reshape(r // group, 128, group * c))

    dwT = np.ascontiguousarray(np.asarray(dense_w, np.float32).T).astype(NBF)
    db_r = np.asarray(dense_b, np.float32).reshape(1, H)
    dwR = pack_rows(dwT, 4)                                         # [2,128,4H]

    f1_eff = np.asarray(fc1_w, np.float32) * np.asarray(ln2_g, np.float32)[None, :]
    b1_eff = np.asarray(fc1_b, np.float32) + np.asarray(fc1_w, np.float32) @ np.asarray(ln2_b, np.float32)
    fc1T = np.ascontiguousarray(f1_eff.T).astype(NBF)               # [H, 4H]
    fc1a = pack_rows(np.ascontiguousarray(fc1T[:, :2 * H]), 2)      # [4,128,4H]
    fc1b = pack_rows(np.ascontiguousarray(fc1T[:, 2 * H:]), 2)      # [4,128,4H]
    b1_t = np.ascontiguousarray(b1_eff.reshape(32, 128).T).astype(np.float32)
    fc2T = np.ascontiguousarray(np.asarray(fc2_w, np.float32).T).astype(NBF)
    fc2R = pack_rows(fc2T, 4)                                       # [8,128,4H]
    b2_r = np.asarray(fc2_b, np.float32).reshape(1, H)
    wqkR = pack_rows(wqkT, 2)                                       # [4,128,4H]
    wvR = pack_rows(wvT, 2)                                         # [4,128,2H]
    return dict(wqkR=wqkR, wvR=wvR, bqkr=bqk_r, bvr=bv_r, db=db_r,
                dwR=dwR, fc1a=fc1a, fc1b=fc1b, b1=b1_t, fc2R=fc2R, b2=b2_r)


def _tri_mask():
    k = np.arange(QB)[:, None]
    q = np.arange(QB)[None, :]
    return np.where(k <= q, 1.0, 0.0).astype(NBF)   # [k, q] allowed k<=q


def kernel(hidden_states, attention_mask, alibi,
           ln1_g, ln1_b, qkv_w, qkv_b, dense_w, dense_b,
           ln2_g, ln2_b, fc1_w, fc1_b, fc2_w, fc2_b):
    X = np.asarray(hidden_states, np.float32).reshape(S, H)
    alibi_np = np.asarray(alibi, np.float32).reshape(NH, S)
    W = _prep_weights(ln1_g, ln1_b, qkv_w, qkv_b, dense_w, dense_b,
                      ln2_g, ln2_b, fc1_w, fc1_b, fc2_w, fc2_b)

    # ---------------- L1 ----------------
    nc1 = _get_nc("l1")
    in1 = []
    xcore = []
    for i in range(NCORE):
        a, b = _blocks(i)
        xi = np.ascontiguousarray(
            np.concatenate([X[a * QB:(a + 1) * QB], X[b * QB:(b + 1) * QB]], 0))
        xcore.append(xi)
        in1.append(dict(x=xi, wqkR=W["wqkR"], wvR=W["wvR"],
                        bqkr=W["bqkr"], bvr=W["bvr"]))
    r1 = _run(nc1, in1)

    # ---------------- host reshuffle ----------------
    KT = np.zeros((H, S), NBF)
    V = np.zeros((S, H), NBF)
    qT_core = []
    for i in range(NCORE):
        a, b = _blocks(i)
        qkS_i = r1[i]["qkS"]                      # [2, 128, 2H] token-major
        qkT_i = np.concatenate([qkS_i[0].T, qkS_i[1].T], axis=1)  # [2H, SC]
        vtm_i = r1[i]["vtS"].reshape(128, 2, H).transpose(1, 0, 2) \
            .reshape(SC, H)
        qT_core.append(qkT_i[:H])
        KT[:, a * QB:(a + 1) * QB] = qkT_i[H:, :QB]
        KT[:, b * QB:(b + 1) * QB] = qkT_i[H:, QB:]
        V[a * QB:(a + 1) * QB] = vtm_i[:QB]
        V[b * QB:(b + 1) * QB] = vtm_i[QB:]

    binm = _tri_mask()
    nc2 = _get_nc("l2")
    in2 = []
    KTf = KT.astype(np.float32).reshape(NH, HD, S)
    Vf = V.astype(np.float32)
    for i in range(NCORE):
        a, bq_ = _blocks(i)
        slots = _slots(i)
        # qaug: per head [66, SC]: rows 0:64 = q^T, row 64 = 1 on A-half,
        # row 65 = 1 on B-half
        qaug = np.zeros((NH, 66, SC), NBF)
        qf = qT_core[i].astype(np.float32).reshape(NH, HD, SC)
        qaug[:, :HD, :] = qf
        qaug[:, HD, :QB] = 1.0
        qaug[:, HD + 1, QB:] = 1.0
        # kaug: per head [66, NSLOT*QB]: rows 0:64 = k^T (permuted chunks),
        # row 64 = alibi + A-prefix mask, row 65 = alibi + B-prefix mask
        kaug = np.zeros((NH, 66, NSLOT * QB), np.float32)
        kaug[:, HD, :] = NEG
        kaug[:, HD + 1, :] = NEG
        for s, c in enumerate(slots):
            if c is None:
                continue
            ck = slice(c * QB, (c + 1) * QB)
            sk = slice(s * QB, (s + 1) * QB)
            kaug[:, :HD, sk] = KTf[:, :, ck]
            av = alibi_np[:, ck]
            kaug[:, HD, sk] = av if c <= a else NEG
            kaug[:, HD + 1, sk] = av
        # vaug: per head [QB, NSLOT*65]: slot s at cols [65s:65s+64] =
        # V chunk slots[s] (keys on partitions), col 65s+64 = 1 (sum row)
        vaug = np.zeros((NH, QB, NSLOT * 65), NBF)
        for s, c in enumerate(slots):
            if c is None:
                continue
            vs = Vf[c * QB:(c + 1) * QB]
            for h in range(NH):
                vaug[h, :, s * 65: s * 65 + HD] = vs[:, h * HD:(h + 1) * HD]
                vaug[h, :, s * 65 + HD] = 1.0
        qaug_p = np.ascontiguousarray(
            qaug.transpose(1, 0, 2).reshape(66, NH * SC))
        in2.append(dict(qaug=qaug_p, kaug=kaug.astype(NBF), vaug=vaug,
                        binm=binm, xres=xcore[i] + W["db"],
                        dwR=W["dwR"], fc1a=W["fc1a"], fc1b=W["fc1b"],
                        b1=W["b1"], fc2R=W["fc2R"], b2=W["b2"]))
    r2 = _run(nc2, in2)

    out = np.zeros((S, H), np.float32)
    for i in range(NCORE):
        a, b = _blocks(i)
        oi = r2[i]["outS"].reshape(128, 2, H).transpose(1, 0, 2) \
            .reshape(SC, H)
        out[a * QB:(a + 1) * QB] = oi[:QB]
        out[b * QB:(b + 1) * QB] = oi[QB:]
    return out.reshape(1, S, H)



# revision 18
# speedup vs baseline: 1.0811x; 1.0811x over previous
"""BloomBlock on 8 TRN2 NeuronCores — 3-launch structure.

  * L1 (data-parallel over tokens): LN1 (folded into weights on host) +
    QKV projection for each core's 256 tokens (blocks i and 15-i).
  * Host: all-gather Q/K/V, regroup per head.
  * L2 (tensor-parallel over heads): each core owns 2 heads for ALL 2048
    queries. Exact-causal attention (no padded key slots): per key chunk
    c, only queries >= 128c are scored. Transposed-score layout (keys on
    partitions, queries on free dim; softmax denominator via an appended
    ones-column on V; alibi via a bias row on K matched with a ones row
    on Q). Diagonal chunks get a post-exp binary stair mask. Fused
    row-parallel dense: each core emits a partial dense output over all
    tokens from its 2 heads' context.
  * Host: reduce dense partials + residual + dense bias -> attn_out.
  * L3 (4 token groups x 2-way tensor-parallel MLP): each core runs LN2
    on its group's 512 tokens and computes fc1/gelu/fc2 for half the 4H
    features; partial fc2 outputs are reduced on host with residual2.
"""

import os
from contextlib import ExitStack

import ml_dtypes
import numpy as np

import concourse.bass as bass
import concourse.tile as tile
from concourse import bacc, mybir
from concourse.masks import make_identity

BF16 = mybir.dt.bfloat16
F32 = mybir.dt.float32
NBF = ml_dtypes.bfloat16

S, H, NH, HD = 2048, 1024, 16, 64
NCORE = 8
QB = 128          # token/key chunk size
SC = 2 * QB       # tokens per core in L1
NSLOT = 16
NHC = H // 128    # hidden chunks
EPS = 1e-5
NORM = float(np.sqrt(HD))  # 8.0 (LAYER_NUMBER = 1)


def _blocks(i):
    return (i, 15 - i)


# ----------------------------------------------------------------------------
# L1: LN1 + QKV, data-parallel over tokens (unchanged from baseline)
# ----------------------------------------------------------------------------

def build_l1():
    nc = bacc.Bacc("TRN2", target_bir_lowering=False, debug=False,
                   num_devices=NCORE)
    x = nc.dram_tensor("x", [SC, H], F32, kind="ExternalInput")
    wqkR = nc.dram_tensor("wqkR", [4, 128, 4 * H], BF16, kind="ExternalInput")
    wvR = nc.dram_tensor("wvR", [4, 128, 2 * H], BF16, kind="ExternalInput")
    bqkr = nc.dram_tensor("bqkr", [1, 2 * H], BF16, kind="ExternalInput")
    bvr = nc.dram_tensor("bvr", [1, H], BF16, kind="ExternalInput")
    qkS = nc.dram_tensor("qkS", [2, 128, 2 * H], BF16, kind="ExternalOutput")
    vtS = nc.dram_tensor("vtS", [128, 2 * H], BF16, kind="ExternalOutput")

    with tile.TileContext(nc) as tc, ExitStack() as ctx:
        singles = ctx.enter_context(tc.tile_pool(name="singles", bufs=1))
        stat = ctx.enter_context(tc.tile_pool(name="stat", bufs=2))
        work = ctx.enter_context(tc.tile_pool(name="work", bufs=2))

        ident = singles.tile([128, 128], BF16)
        make_identity(nc, ident)
        epst = singles.tile([128, 1], F32)
        nc.vector.memset(epst, EPS)
        warm = singles.tile([1, 1], F32)
        nc.scalar.activation(out=warm, in_=epst[0:1, 0:1],
                             func=mybir.ActivationFunctionType.Sqrt,
                             bias=0.0, scale=1.0)
        x_half = []
        for t in range(2):
            for hh in range(2):
                xt = singles.tile([128, 512], F32, tag=f"x{t}{hh}")
                nc.sync.dma_start(
                    out=xt, in_=x.ap()[t * 128:(t + 1) * 128,
                                       hh * 512:(hh + 1) * 512])
                x_half.append(xt)
        wqk_sb = []
        for g in range(4):
            wt = singles.tile([128, 4 * H], BF16, tag=f"wqk{g}")
            nc.sync.dma_start(out=wt, in_=wqkR.ap()[g])
            wqk_sb.append(wt)
        wv_sb = []
        for g in range(4):
            wt = singles.tile([128, 2 * H], BF16, tag=f"wv{g}")
            nc.sync.dma_start(out=wt, in_=wvR.ap()[g])
            wv_sb.append(wt)
        bqk_sb = singles.tile([128, 2 * H], BF16)
        nc.sync.dma_start(out=bqk_sb, in_=bqkr.ap().to_broadcast([128, 2 * H]))
        bv_sb = singles.tile([128, H], BF16)
        nc.sync.dma_start(out=bv_sb, in_=bvr.ap().to_broadcast([128, H]))
        qk_stage = singles.tile([128, 2, 2 * H], BF16)
        vt_stage = singles.tile([128, 2 * H], BF16)

        xhatT = []
        for c in range(NHC):
            xc = singles.tile([128, SC], BF16, tag=f"xhatT{c}")
            xhatT.append(xc)

        def _ln_tile(t):
            st = stat.tile([128, 2, 6], F32, tag="bnst")
            nc.vector.bn_stats(out=st[:, 0, :], in_=x_half[2 * t])
            nc.vector.bn_stats(out=st[:, 1, :], in_=x_half[2 * t + 1])
            mv = stat.tile([128, 2], F32, tag="bnmv")
            nc.vector.bn_aggr(out=mv, in_=st)
            rstd = stat.tile([128, 1], F32, tag="rstd")
            nc.scalar.activation(out=rstd, in_=mv[:, 1:2],
                                 func=mybir.ActivationFunctionType.Sqrt,
                                 bias=epst, scale=1.0)
            nc.vector.reciprocal(out=rstd, in_=rstd)
            xh = work.tile([128, H], BF16, tag="xhat")
            for hh in range(2):
                nc.vector.tensor_scalar(
                    out=xh[:, hh * 512:(hh + 1) * 512],
                    in0=x_half[2 * t + hh],
                    scalar1=mv[:, 0:1], scalar2=rstd,
                    op0=mybir.AluOpType.subtract,
                    op1=mybir.AluOpType.mult)
            return xh

        def _tp_tile(tp_ps, t, xh):
            for c in range(NHC):
                pst = tp_ps.tile([128, 128], BF16, tag="tp")
                nc.tensor.transpose(pst, xh[:, c * 128:(c + 1) * 128],
                                    ident)
                nc.vector.tensor_copy(
                    out=xhatT[c][:, t * 128:(t + 1) * 128], in_=pst)

        with tc.tile_pool(name="tp_ps", bufs=2, space="PSUM") as tp_ps:
            xh0 = _ln_tile(0)
            xh1 = _ln_tile(1)
            _tp_tile(tp_ps, 0, xh0)
            _tp_tile(tp_ps, 1, xh1)

        with tc.tile_pool(name="qk_ps", bufs=1, space="PSUM") as qk_ps:
            for t in range(2):
                psqk = qk_ps.tile([128, 2 * H], F32, tag=f"psqk{t}")
                for c in range(NHC):
                    wt = wqk_sb[c // 2]
                    for n in range(4):
                        nc.tensor.matmul(
                            psqk[:, n * 512:(n + 1) * 512],
                            lhsT=xhatT[c][:, t * 128:(t + 1) * 128],
                            rhs=wt[:, (c % 2) * 2 * H + n * 512:
                                   (c % 2) * 2 * H + (n + 1) * 512],
                            start=(c == 0), stop=(c == NHC - 1))
                for n in range(2):
                    sl = slice(n * 1024, (n + 1) * 1024)
                    nc.vector.tensor_add(out=qk_stage[:, t, sl],
                                         in0=psqk[:, sl], in1=bqk_sb[:, sl])
                nc.sync.dma_start(out=qkS.ap()[t], in_=qk_stage[:, t, :])

            for t in range(2):
                psv = qk_ps.tile([128, 2 * H], F32, tag=f"psqk{t}")
                for c in range(NHC):
                    wt = wv_sb[c // 2]
                    for n in range(2):
                        nc.tensor.matmul(
                            psv[:, n * 512:(n + 1) * 512],
                            lhsT=xhatT[c][:, t * 128:(t + 1) * 128],
                            rhs=wt[:, (c % 2) * H + n * 512:
                                   (c % 2) * H + (n + 1) * 512],
                            start=(c == 0), stop=(c == NHC - 1))
                nc.vector.tensor_add(out=vt_stage[:, t * H:(t + 1) * H],
                                     in0=psv[:, 0:H], in1=bv_sb)
            nc.sync.dma_start(out=vtS.ap(), in_=vt_stage)
    nc.compile()
    return nc


# ----------------------------------------------------------------------------
# L2: exact-causal attention, tensor-parallel over heads (2 heads/core),
#     fused row-parallel dense partial.
# ----------------------------------------------------------------------------

def build_l2():
    nc = bacc.Bacc("TRN2", target_bir_lowering=False, debug=False,
                   num_devices=NCORE)
    # per head: qaug rows 0:64 = q^T/NORM, row 64 = ones
    #           kaug rows 0:64 = k^T,      row 64 = alibi
    #           vaug slot c cols [65c:65c+64) = V chunk c, col 65c+64 = ones
    qaugD = nc.dram_tensor("qaug", [2, 65, S], BF16, kind="ExternalInput")
    kaugD = nc.dram_tensor("kaug", [2, 65, S], BF16, kind="ExternalInput")
    vaugD = nc.dram_tensor("vaug", [2, 128, 16 * 65], BF16, kind="ExternalInput")
    binmD = nc.dram_tensor("binm", [QB, QB], BF16, kind="ExternalInput")
    dwD = nc.dram_tensor("dw", [128, H], BF16, kind="ExternalInput")
    # dense partial, token-major packed: [p, 1024*t + f] = token 128t+p
    outD = nc.dram_tensor("outD", [128, 16 * H], BF16, kind="ExternalOutput")

    with tile.TileContext(nc) as tc, ExitStack() as ctx:
        singles = ctx.enter_context(tc.tile_pool(name="singles", bufs=1))
        kv = ctx.enter_context(tc.tile_pool(name="kv", bufs=2))
        probs = ctx.enter_context(tc.tile_pool(name="probs", bufs=3))
        work = ctx.enter_context(tc.tile_pool(name="work", bufs=2))
        outp = ctx.enter_context(tc.tile_pool(name="outp", bufs=3))

        binm = singles.tile([QB, QB], BF16)
        dw = singles.tile([128, H], BF16)
        ctxT = singles.tile([128, S], BF16)
        epst = singles.tile([1, 1], F32)
        nc.vector.memset(epst, EPS)
        warm = singles.tile([1, 1], F32)
        nc.scalar.activation(out=warm, in_=epst,
                             func=mybir.ActivationFunctionType.Exp,
                             bias=0.0, scale=1.0)

        with tc.tile_pool(name="ps", bufs=1, space="PSUM") as pp:

            def _dense_tile(t, use_act=False):
                psd = pp.tile([128, 1024], F32, tag="sc", bufs=2, name="psd")
                for n in range(2):
                    nc.tensor.matmul(
                        psd[:, n * 512:(n + 1) * 512],
                        lhsT=ctxT[:, t * 128:(t + 1) * 128],
                        rhs=dw[:, n * 512:(n + 1) * 512],
                        start=True, stop=True)
                stg = outp.tile([128, 1024], BF16, tag="og")
                if use_act:
                    nc.scalar.activation(
                        out=stg, in_=psd,
                        func=mybir.ActivationFunctionType.Copy,
                        bias=0.0, scale=1.0)
                else:
                    nc.vector.tensor_copy(out=stg, in_=psd)
                nc.sync.dma_start(out=outD.ap()[:, t * H:(t + 1) * H],
                                  in_=stg)

            def _scores(kt, qt, c, qa, qb):
                """scores + exp + diag mask; returns probs tile."""
                base = 128 * c
                n = qb - qa
                ps = pp.tile([128, 1024], F32, tag="sc", bufs=2, name="ps")
                for s in range(0, n, 512):
                    sl = min(512, n - s)
                    nc.tensor.matmul(ps[:, s:s + sl],
                                     lhsT=kt[:, base:base + 128],
                                     rhs=qt[:, qa + s:qa + s + sl],
                                     start=True, stop=True)
                pb = probs.tile([128, 1024], BF16, tag="pb")
                nc.scalar.activation(out=pb[:, 0:n], in_=ps[:, 0:n],
                                     func=mybir.ActivationFunctionType.Exp,
                                     bias=0.0, scale=1.0)
                if qa == base:  # window starts at the diagonal chunk
                    nc.vector.tensor_mul(out=pb[:, 0:128], in0=pb[:, 0:128],
                                         in1=binm)
                return pb

            def _ctx(vt, pc, c, qa, qb, pb):
                n = qb - qa
                slices = []
                if qa == 128 * c:
                    slices.append((0, 128, True))
                    p = 128
                else:
                    p = 0
                while p < n:
                    nxt = min(n, ((qa + p) // 512 + 1) * 512 - qa)
                    slices.append((p, nxt - p, False))
                    p = nxt
                off = qa if qa < 1024 else qa - 1024
                for (s, sl, stp) in slices:
                    nc.tensor.matmul(pc[:, off + s:off + s + sl],
                                     lhsT=vt[:, 65 * c:65 * c + 65],
                                     rhs=pb[:, s:s + sl],
                                     start=(c == 0), stop=stp,
                                     skip_group_check=True)

            def _normalize(pc_sl, h, dst0, n):
                rec = work.tile([1, 1024], F32, tag="rec")
                nc.vector.reciprocal(out=rec[:, 0:n], in_=pc_sl[64:65, :])
                recb = work.tile([64, 1024], F32, tag="recb")
                nc.gpsimd.partition_broadcast(recb[:, 0:n], rec[0:1, 0:n],
                                              channels=64)
                nc.vector.tensor_mul(
                    out=ctxT[64 * h:64 * h + 64, dst0:dst0 + n],
                    in0=pc_sl[0:64, :], in1=recb[:, 0:n])

            # streams split so the first windows of BOTH heads start early
            kt, qt, vt = [], [], []
            for h in range(2):
                kt.append(kv.tile([65, S], BF16, tag=f"k{h}", bufs=1,
                                  name=f"kt{h}"))
                qt.append(kv.tile([65, S], BF16, tag=f"q{h}", bufs=1,
                                  name=f"qt{h}"))
                vt.append(kv.tile([128, 16 * 65], BF16, tag=f"v{h}", bufs=1,
                                  name=f"vt{h}"))
            for h in range(2):
                nc.sync.dma_start(out=kt[h][:, 0:512],
                                  in_=kaugD.ap()[h][:, 0:512])
                nc.sync.dma_start(out=qt[h][:, 0:1024],
                                  in_=qaugD.ap()[h][:, 0:1024])
            for h in range(2):
                nc.sync.dma_start(out=vt[h][:, 0:520],
                                  in_=vaugD.ap()[h][:, 0:520])
            nc.sync.dma_start(out=binm, in_=binmD.ap())
            for h in range(2):
                nc.sync.dma_start(out=kt[h][:, 512:S],
                                  in_=kaugD.ap()[h][:, 512:S])
                nc.sync.dma_start(out=qt[h][:, 1024:S],
                                  in_=qaugD.ap()[h][:, 1024:S])
            for h in range(2):
                nc.sync.dma_start(out=vt[h][:, 520:16 * 65],
                                  in_=vaugD.ap()[h][:, 520:16 * 65])
            nc.sync.dma_start(out=dw, in_=dwD.ap())

            # both heads phase-interleaved: A sweeps (queries [128c,1024)),
            # then B sweeps ([1024,2048)); pctx psum regions are reused
            # between phases (tag rotation inserts the WAR dep).
            pcA = [pp.tile([65, 1024], F32, tag=f"pc{h}", bufs=1,
                           name=f"pcA{h}") for h in range(2)]
            wins = [(h, 0, c, 128 * c, 1024)
                    for c in range(8) for h in range(2)]
            pend = None
            nd = 0
            for wi, (h, phase, c, qa, qb) in enumerate(list(wins)):
                pb = _scores(kt[h], qt[h], c, qa, qb)
                if pend is not None:
                    _ctx(*pend)
                pend = (vt[h], pcA[h], c, qa, qb, pb)
            pcB = [pp.tile([65, 1024], F32, tag=f"pc{h}", bufs=1,
                           name=f"pcB{h}") for h in range(2)]
            winsB = [(h, 1, c, (1024 if c < 8 else 128 * c), 2048)
                     for c in range(16) for h in range(2)]
            avail = 0
            for wi, (h, phase, c, qa, qb) in enumerate(winsB):
                pb = _scores(kt[h], qt[h], c, qa, qb)
                if pend is not None:
                    _ctx(*pend)
                pend = (vt[h], pcB[h], c, qa, qb, pb)
                # dense scheduling: DVE copies only in the mask-free half
                # (wi<16, chunks c<8 have no diagonal), Act copies in the
                # Act-light shrinking-window region (wi>=26)
                if wi < 16:
                    if wi % 2 == 1 and wi >= 3 and nd < min(avail, 8):
                        _dense_tile(nd, use_act=False)
                        nd += 1
                elif wi >= 26 and nd < avail:
                    _dense_tile(nd, use_act=True)
                    nd += 1
                # hooks AFTER the dense call: avail takes effect next window
                if wi == 1:
                    # last A ctx flushed; both A halves complete
                    _normalize(pcA[0], 0, 0, 1024)
                    _normalize(pcA[1], 1, 0, 1024)
                    avail = 8
                if wi == 24:
                    # chunks 8..11 flushed: abs queries [1024,1536) final
                    _normalize(pcB[0][:, 0:512], 0, 1024, 512)
                    _normalize(pcB[1][:, 0:512], 1, 1024, 512)
                    avail = 12
                if wi in (26, 28, 30):
                    k = (wi - 2) // 2 - 8   # 4, 5, 6
                    _normalize(pcB[0][:, 128 * k:128 * (k + 1)],
                               0, 1024 + 128 * k, 128)
                    _normalize(pcB[1][:, 128 * k:128 * (k + 1)],
                               1, 1024 + 128 * k, 128)
                    avail = 9 + k
            _ctx(*pend)
            if nd < 15:
                _dense_tile(nd, use_act=True)
                nd += 1
            _normalize(pcB[0][:, 896:1024], 0, 1920, 128)
            _normalize(pcB[1][:, 896:1024], 1, 1920, 128)
            while nd < 16:
                _dense_tile(nd, use_act=(nd % 2 == 1))
                nd += 1
    nc.compile()
    return nc


# ----------------------------------------------------------------------------
# L3: LN2 + MLP, 4 token groups x 2-way tensor-parallel over 4H
# ----------------------------------------------------------------------------

def build_l3():
    nc = bacc.Bacc("TRN2", target_bir_lowering=False, debug=False,
                   num_devices=NCORE)
    # xres: [p, 1024t + f] = attn_out token 128t+p (512 tokens per core)
    xresD = nc.dram_tensor("xres", [128, 4 * H], BF16, kind="ExternalInput")
    # f1T: m-major packing: [p, 1024*m + 128*c + j] = fc1_eff.T[128c+p, 2048*half + 128m + j]
    f1TD = nc.dram_tensor("f1T", [4, 128, 4 * H], BF16, kind="ExternalInput")
    b1D = nc.dram_tensor("b1c", [128, 16], F32, kind="ExternalInput")
    # f2T: chunk m at cols [1024m:1024(m+1)): fc2_w.T[2048*half+128m+p, f]
    f2TD = nc.dram_tensor("f2T", [4, 128, 4 * H], BF16, kind="ExternalInput")
    outP = nc.dram_tensor("outP", [128, 4 * H], BF16, kind="ExternalOutput")

    with tile.TileContext(nc) as tc, ExitStack() as ctx:
        singles = ctx.enter_context(tc.tile_pool(name="singles", bufs=1))
        stat = ctx.enter_context(tc.tile_pool(name="stat", bufs=2))
        work = ctx.enter_context(tc.tile_pool(name="work", bufs=2))
        hp = ctx.enter_context(tc.tile_pool(name="hp", bufs=1))
        outp = ctx.enter_context(tc.tile_pool(name="outp", bufs=2))

        ident = singles.tile([128, 128], BF16)
        make_identity(nc, ident)
        epst = singles.tile([128, 1], F32)
        nc.vector.memset(epst, EPS)
        warm = singles.tile([1, 1], F32)
        nc.scalar.activation(out=warm, in_=epst[0:1, 0:1],
                             func=mybir.ActivationFunctionType.Sqrt,
                             bias=0.0, scale=1.0)

        xres = singles.tile([128, 4, H], BF16)
        b1c = singles.tile([128, 16], F32)
        f1 = [singles.tile([128, 4 * H], BF16, tag=f"f1{g}", name=f"f1{g}")
              for g in range(4)]
        f2 = [singles.tile([128, 4 * H], BF16, tag=f"f2{g}", name=f"f2{g}")
              for g in range(4)]
        # interleave: first two xres tiles (gate LN2 for fc1's first half),
        # then the first fc1 group, then the rest
        nc.sync.dma_start(out=xres[:, 0, :], in_=xresD.ap()[:, 0:H])
        nc.sync.dma_start(out=xres[:, 1, :], in_=xresD.ap()[:, H:2 * H])
        nc.sync.dma_start(out=b1c, in_=b1D.ap())
        nc.sync.dma_start(out=f1[0], in_=f1TD.ap()[0])
        nc.sync.dma_start(out=xres[:, 2, :], in_=xresD.ap()[:, 2 * H:3 * H])
        nc.sync.dma_start(out=xres[:, 3, :], in_=xresD.ap()[:, 3 * H:4 * H])
        for g in range(1, 4):
            nc.sync.dma_start(out=f1[g], in_=f1TD.ap()[g])
        for g in range(4):
            nc.sync.dma_start(out=f2[g], in_=f2TD.ap()[g])

        # xh2T: [p, c, 128t+j] = xhat^T chunk c
        xh2T = singles.tile([128, NHC, 512], BF16)

        with tc.tile_pool(name="ps", bufs=1, space="PSUM") as pp:
            for t in range(4):
                st = stat.tile([128, 2, 6], F32, tag="bnst")
                nc.vector.bn_stats(out=st[:, 0, :], in_=xres[:, t, 0:512])
                nc.vector.bn_stats(out=st[:, 1, :], in_=xres[:, t, 512:1024])
                mv = stat.tile([128, 2], F32, tag="bnmv")
                nc.vector.bn_aggr(out=mv, in_=st)
                rstd = stat.tile([128, 1], F32, tag="rstd")
                nc.scalar.activation(out=rstd, in_=mv[:, 1:2],
                                     func=mybir.ActivationFunctionType.Sqrt,
                                     bias=epst, scale=1.0)
                nc.vector.reciprocal(out=rstd, in_=rstd)
                xh = work.tile([128, H], BF16, tag="xhat")
                nc.vector.tensor_scalar(out=xh, in0=xres[:, t, :],
                                        scalar1=mv[:, 0:1], scalar2=rstd,
                                        op0=mybir.AluOpType.subtract,
                                        op1=mybir.AluOpType.mult)
                tp = pp.tile([128, 1024], BF16, tag="tp", bufs=1)
                for c in range(NHC):
                    nc.tensor.transpose(tp[:, c * 128:(c + 1) * 128],
                                        xh[:, c * 128:(c + 1) * 128], ident)
                nc.vector.tensor_copy(out=xh2T[:, :, t * 128:(t + 1) * 128],
                                      in_=tp)

            hts = {}

            def _fc1(m):
                ps = pp.tile([128, 512], F32, tag="f1", bufs=3, name="psf1")
                # token-halves so the first fc1 only waits on 2 LN2 tiles
                for tg in range(2):
                    for c in range(NHC):
                        nc.tensor.matmul(
                            ps[:, tg * 256:(tg + 1) * 256],
                            lhsT=f1[m // 4][:, (m % 4) * 1024 + c * 128:
                                            (m % 4) * 1024 + (c + 1) * 128],
                            rhs=xh2T[:, c, tg * 256:(tg + 1) * 256],
                            start=(c == 0), stop=(c == NHC - 1))
                ht = hp.tile([128, 512], BF16, tag=f"h{m}")
                if os.environ.get("BLOOM_SIM"):
                    u = work.tile([128, 512], F32, tag="gelu_u")
                    nc.vector.tensor_scalar_add(out=u, in0=ps,
                                                scalar1=b1c[:, m:m + 1])
                    s2 = work.tile([128, 512], F32, tag="gelu_s")
                    nc.vector.tensor_mul(out=s2, in0=u, in1=u)
                    nc.vector.tensor_scalar(out=s2, in0=s2,
                                            scalar1=0.035677408145115,
                                            scalar2=0.7978845608028654,
                                            op0=mybir.AluOpType.mult,
                                            op1=mybir.AluOpType.add)
                    nc.vector.tensor_mul(out=s2, in0=s2, in1=u)
                    nc.scalar.activation(out=s2, in_=s2,
                                         func=mybir.ActivationFunctionType.Tanh,
                                         bias=0.0, scale=1.0)
                    nc.vector.tensor_scalar(out=s2, in0=s2, scalar1=1.0,
                                            scalar2=0.5,
                                            op0=mybir.AluOpType.add,
                                            op1=mybir.AluOpType.mult)
                    nc.vector.tensor_mul(out=ht, in0=s2, in1=u)
                else:
                    nc.scalar.activation(
                        out=ht, in_=ps,
                        func=mybir.ActivationFunctionType.Gelu_apprx_tanh,
                        bias=b1c[:, m:m + 1], scale=1.0)
                hts[m] = ht

            def _fc2(psf2, m, tpair):
                ht = hts[m]
                for ti, t in enumerate(tpair):
                    for n in range(2):
                        nc.tensor.matmul(
                            psf2[ti][:, n * 512:(n + 1) * 512],
                            lhsT=ht[:, t * 128:(t + 1) * 128],
                            rhs=f2[m // 4][:, (m % 4) * 1024 + n * 512:
                                           (m % 4) * 1024 + (n + 1) * 512],
                            start=(m == 0), stop=(m == 15))

            def _drain(psf2, tpair):
                for ti, t in enumerate(tpair):
                    stg = outp.tile([128, 1024], BF16, tag="og")
                    if t % 2 == 0:
                        nc.vector.tensor_copy(out=stg, in_=psf2[ti])
                    else:
                        nc.scalar.activation(
                            out=stg, in_=psf2[ti],
                            func=mybir.ActivationFunctionType.Copy,
                            bias=0.0, scale=1.0)
                    nc.sync.dma_start(out=outP.ap()[:, t * H:(t + 1) * H],
                                      in_=stg)

            # pass 1: fc1 all m, fc2 into token tiles 0,1 (staggered)
            psf2 = [pp.tile([128, 1024], F32, tag=f"f2_{t}", bufs=1,
                            name=f"psf2_{t}")
                    for t in range(2)]
            _fc1(0)
            for m in range(16):
                if m + 1 < 16:
                    _fc1(m + 1)
                _fc2(psf2, m, (0, 1))
            _drain(psf2, (0, 1))
            # passes 2/3: fc2 for token tiles 2 then 3 (staggered drains)
            psf2b = [pp.tile([128, 1024], F32, tag="f2_0", bufs=1,
                             name="psf2b")]
            for m in range(16):
                _fc2(psf2b, m, (2,))
            _drain(psf2b, (2,))
            psf2c = [pp.tile([128, 1024], F32, tag="f2_1", bufs=1,
                             name="psf2c")]
            for m in range(16):
                _fc2(psf2c, m, (3,))
            _drain(psf2c, (3,))
    nc.compile()
    return nc


# ----------------------------------------------------------------------------
# host orchestration
# ----------------------------------------------------------------------------

_NC_CACHE = {}
_BUILDERS = {"l1": build_l1, "l2": build_l2, "l3": build_l3}


def _get_nc(name):
    if name not in _NC_CACHE:
        _NC_CACHE[name] = _BUILDERS[name]()
    return _NC_CACHE[name]


def _run(nc, in_maps):
    if os.environ.get("BLOOM_SIM"):
        from concourse.bass_interp import CoreSim
        results = []
        for m in in_maps:
            sim = CoreSim(nc, trace=False)
            for k, v in m.items():
                sim.tensor(k)[:] = v
            sim.simulate(check_with_hw=False)
            outs = {}
            for alloc in nc.m.functions[0].allocations:
                if getattr(alloc, "kind", None) == "ExternalOutput":
                    nm = alloc.memorylocations[0].name
                    outs[nm] = np.array(sim.tensor(nm))
            results.append(outs)
        return results
    from concourse.bass_utils import run_bass_kernel_spmd
    res = run_bass_kernel_spmd(nc, in_maps, core_ids=list(range(NCORE)))
    return res.results


def _prep_weights(ln1_g, ln1_b, qkv_w, qkv_b, dense_w, dense_b,
                  ln2_g, ln2_b, fc1_w, fc1_b, fc2_w, fc2_b):
    qkv_w = np.asarray(qkv_w, np.float32)
    qkv_b = np.asarray(qkv_b, np.float32)
    w_eff = qkv_w * np.asarray(ln1_g, np.float32)[None, :]
    b_eff = qkv_b + qkv_w @ np.asarray(ln1_b, np.float32)
    w3 = w_eff.reshape(NH, 3 * HD, H)
    b3 = b_eff.reshape(NH, 3 * HD)
    wq = w3[:, :HD, :] / NORM
    wk = w3[:, HD:2 * HD, :]
    wv = w3[:, 2 * HD:, :]
    bq = b3[:, :HD] / NORM
    bk = b3[:, HD:2 * HD]
    bv = b3[:, 2 * HD:]
    wqk = np.concatenate([wq.reshape(H, H), wk.reshape(H, H)], 0)  # [2H, H]
    wqkT = np.ascontiguousarray(wqk.T).astype(NBF)                  # [H, 2H]
    wvT = np.ascontiguousarray(wv.reshape(H, H).T).astype(NBF)      # [H, H]
    bqk = np.concatenate([bq.reshape(H), bk.reshape(H)])            # [2H]
    bqk_r = np.ascontiguousarray(bqk.reshape(1, 2 * H)).astype(NBF)
    bv_r = np.ascontiguousarray(bv.reshape(1, H)).astype(NBF)

    def pack_rows(w, group):
        r, c = w.shape[0] // 128, w.shape[1]
        return np.ascontiguousarray(
            w.reshape(r // group, group, 128, c).transpose(0, 2, 1, 3)
            .reshape(r // group, 128, group * c))

    dwT = np.ascontiguousarray(np.asarray(dense_w, np.float32).T).astype(NBF)
    db_r = np.asarray(dense_b, np.float32).reshape(1, H)

    f1_eff = np.asarray(fc1_w, np.float32) * np.asarray(ln2_g, np.float32)[None, :]
    b1_eff = np.asarray(fc1_b, np.float32) + np.asarray(fc1_w, np.float32) @ np.asarray(ln2_b, np.float32)
    fc1T = np.ascontiguousarray(f1_eff.T)                           # [H, 4H]
    fc2T = np.ascontiguousarray(np.asarray(fc2_w, np.float32).T)    # [4H, H]
    b2_r = np.asarray(fc2_b, np.float32).reshape(1, H)
    wqkR = pack_rows(wqkT, 2)                                       # [4,128,4H]
    wvR = pack_rows(wvT, 2)                                         # [4,128,2H]

    # L3 packings, per half
    f1T_half, b1_half, f2T_half = [], [], []
    for half in range(2):
        cols = slice(half * 2 * H, (half + 1) * 2 * H)
        f1h = fc1T[:, cols]                                         # [1024, 2048]
        # f1TD[g, p, 1024*(m%4) + 128c + j] = f1h[128c + p, 128m + j]
        a = (f1h.reshape(NHC, 128, 16, 128)      # [c, p, m, j]
             .transpose(2, 1, 0, 3)              # [m, p, c, j]
             .reshape(4, 4, 128, NHC * 128)      # [g, m%4, p, c*j]
             .transpose(0, 2, 1, 3)              # [g, p, m%4, c*j]
             .reshape(4, 128, 4 * H))
        f1T_half.append(np.ascontiguousarray(a.astype(NBF)))
        b1h = b1_eff[half * 2 * H:(half + 1) * 2 * H]
        b1_half.append(np.ascontiguousarray(
            b1h.reshape(16, 128).T).astype(np.float32))
        f2h = fc2T[half * 2 * H:(half + 1) * 2 * H, :]              # [2048, 1024]
        # f2TD[g, p, 1024*(m%4) + f] = f2h[128m + p, f]
        b = (f2h.reshape(4, 4, 128, H)           # [g, m%4, p, f]
             .transpose(0, 2, 1, 3)              # [g, p, m%4, f]
             .reshape(4, 128, 4 * H))
        f2T_half.append(np.ascontiguousarray(b.astype(NBF)))
    return dict(wqkR=wqkR, wvR=wvR, bqkr=bqk_r, bvr=bv_r, db=db_r,
                dwT=dwT, f1T_half=f1T_half, b1_half=b1_half,
                f2T_half=f2T_half, b2=b2_r)


def _tri_mask():
    k = np.arange(QB)[:, None]
    q = np.arange(QB)[None, :]
    return np.where(k <= q, 1.0, 0.0).astype(NBF)   # [k, q] allowed k<=q


def kernel(hidden_states, attention_mask, alibi,
           ln1_g, ln1_b, qkv_w, qkv_b, dense_w, dense_b,
           ln2_g, ln2_b, fc1_w, fc1_b, fc2_w, fc2_b):
    X = np.asarray(hidden_states, np.float32).reshape(S, H)
    alibi_np = np.asarray(alibi, np.float32).reshape(NH, S)
    W = _prep_weights(ln1_g, ln1_b, qkv_w, qkv_b, dense_w, dense_b,
                      ln2_g, ln2_b, fc1_w, fc1_b, fc2_w, fc2_b)

    # ---------------- L1 ----------------
    nc1 = _get_nc("l1")
    in1 = []
    for i in range(NCORE):
        a, b = _blocks(i)
        xi = np.ascontiguousarray(
            np.concatenate([X[a * QB:(a + 1) * QB], X[b * QB:(b + 1) * QB]], 0))
        in1.append(dict(x=xi, wqkR=W["wqkR"], wvR=W["wvR"],
                        bqkr=W["bqkr"], bvr=W["bvr"]))
    r1 = _run(nc1, in1)

    # ---------------- host gather: full Q^T, K^T, V ----------------
    QT = np.zeros((H, S), NBF)
    KT = np.zeros((H, S), NBF)
    V = np.zeros((S, H), NBF)
    for i in range(NCORE):
        a, b = _blocks(i)
        qkS_i = r1[i]["qkS"]                      # [2, 128, 2H] token-major
        qkT_i = np.concatenate([qkS_i[0].T, qkS_i[1].T], axis=1)  # [2H, SC]
        vtm_i = r1[i]["vtS"].reshape(128, 2, H).transpose(1, 0, 2) \
            .reshape(SC, H)
        QT[:, a * QB:(a + 1) * QB] = qkT_i[:H, :QB]
        QT[:, b * QB:(b + 1) * QB] = qkT_i[:H, QB:]
        KT[:, a * QB:(a + 1) * QB] = qkT_i[H:, :QB]
        KT[:, b * QB:(b + 1) * QB] = qkT_i[H:, QB:]
        V[a * QB:(a + 1) * QB] = vtm_i[:QB]
        V[b * QB:(b + 1) * QB] = vtm_i[QB:]

    # ---------------- L2 ----------------
    binm = _tri_mask()
    dwT = W["dwT"]                                 # [H, H] f32 -> slices
    nc2 = _get_nc("l2")
    in2 = []
    Vf = V.astype(np.float32)
    for i in range(NCORE):
        qaug = np.zeros((2, 65, S), NBF)
        kaug = np.zeros((2, 65, S), NBF)
        vaug = np.zeros((2, 128, 16 * 65), NBF)
        for j in range(2):
            h = 2 * i + j
            qaug[j, :HD, :] = QT[h * HD:(h + 1) * HD, :]
            qaug[j, HD, :] = 1.0
            kaug[j, :HD, :] = KT[h * HD:(h + 1) * HD, :]
            kaug[j, HD, :] = alibi_np[h].astype(NBF)
            for c in range(16):
                vaug[j, :, 65 * c:65 * c + HD] = \
                    Vf[c * QB:(c + 1) * QB, h * HD:(h + 1) * HD]
                vaug[j, :, 65 * c + HD] = 1.0
        dwi = np.ascontiguousarray(dwT[i * 128:(i + 1) * 128, :]).astype(NBF)
        in2.append(dict(qaug=qaug, kaug=kaug, vaug=vaug, binm=binm, dw=dwi))
    r2 = _run(nc2, in2)

    # host reduce: attn_out = sum of dense partials + residual + dense bias
    attn_out = X + W["db"]
    for i in range(NCORE):
        attn_out = attn_out + r2[i]["outD"].astype(np.float32) \
            .reshape(128, 16, H).transpose(1, 0, 2).reshape(S, H)

    # ---------------- L3 ----------------
    nc3 = _get_nc("l3")
    in3 = []
    attn_bf = attn_out.astype(NBF)
    for i in range(NCORE):
        g, half = i // 2, i % 2
        xg = attn_bf[512 * g:512 * (g + 1)]        # [512, H]
        xres = np.ascontiguousarray(
            xg.reshape(4, 128, H).transpose(1, 0, 2).reshape(128, 4 * H))
        in3.append(dict(xres=xres, f1T=W["f1T_half"][half],
                        b1c=W["b1_half"][half], f2T=W["f2T_half"][half]))
    r3 = _run(nc3, in3)

    out = np.empty((S, H), np.float32)
    for g in range(4):
        p = r3[2 * g]["outP"].astype(np.float32) + \
            r3[2 * g + 1]["outP"].astype(np.float32)
        out[512 * g:512 * (g + 1)] = \
            p.reshape(128, 4, H).transpose(1, 0, 2).reshape(512, H) \
            + attn_out[512 * g:512 * (g + 1)] + W["b2"]
    return out.reshape(1, S, H)
